# revision 1
# baseline (speedup 1.0000x reference)
"""DeepONet+GRU Trainium2 kernel v2 (8-core data parallel).

Full inputs in, full outputs out. Batch 1024 sharded 128/core; params
replicated. Per core:
  branch MLP (528->512 x4) -> branchT [512f, 128b]   (as baseline)
  trunk MLP (1->512 x4)    -> trunkT  [512f, 128t]
  seq = branchT.T @ trunkT -> saug rows (t-major, bf16)
  2-layer GRU scan, hidden 256, computed entirely in TRANSPOSED
  [feature, batch] layout: state h01 [128, 512] bf16 holds
  [h0_f0 | h0_f1 | h1_f0 | h1_f1]. Recurrent/input weights are
  pre-transposed host-side into matmul lhsT blocks, so each step is a
  flat stream of [128,128] matmuls with no PE transposes and no
  PSUM->SBUF state copies; gate math uses bf16 DVE ops (2x/4x modes)
  and per-step biases ride the K=2/K=1 matmuls.
  L1 runs one step behind L0 so its chain overlaps L0's.
"""
import sys
sys.path.insert(0, '/opt/trn_rl_repo')

import numpy as np

B = 1024
BC = 128          # batch per core
NB = 528
NBP = 640         # padded branch input (5 k-tiles)
HID = 512
GH = 256
T = 128
NS = 16
NCORES = 8
N_PAD = 2

_CACHE = {}


def _patched_tile_context(nc):
    """TileContext whose tail drain splits sem waits (walrus CoreV3 rejects
    >1 sync wait on a Drain)."""
    import concourse.tile as tile
    from concourse.vector_clock import ScopedClock

    class PatchedTileContext(tile.TileContext):
        def _drain_and_barrier(self, tick_clock, wait_clock):
            nc = self.nc
            drain_inst = nc.sync.drain()
            wait_clock.add_sem_waits(
                drain_inst.ins, ScopedClock({None: tick_clock.global_clock})
            )
            si = drain_inst.ins.sync_info
            waits = list(si.on_wait or []) if si is not None else []
            if len(waits) > 1:
                si.on_wait = waits[:1]
                for i in range(1, len(waits)):
                    extra = nc.sync.drain()
                    esi = extra.ins.sync_info
                    if esi is None:
                        from concourse import mybir
                        extra.ins.sync_info = mybir.SyncInfo(
                            on_wait=waits[i:i + 1], on_update=[]
                        )
                    else:
                        esi.on_wait = waits[i:i + 1]
            nc.all_engine_barrier()
            assert self.sems is not None
            popped = nc._tile_sem_poison_stack.pop()
            assert popped is self._sem_poison
            nc.clear_and_free_semaphores(list(self.sems.allocated().values()))
            nc.all_engine_barrier()

    return PatchedTileContext(nc)


def _split_multi_waits(nc):
    """This container's walrus rejects >1 sync wait per instruction.
    Hoist extra waits onto engine-matched NoOps spliced immediately before
    the offending instruction."""
    from concourse import mybir
    n_extra = 0
    for fn in nc.m.functions:
        for bb in fn.blocks:
            new = []
            for inst in bb.instructions:
                si = inst.sync_info
                waits = list(si.on_wait) if (si is not None and si.on_wait) else []
                if len(waits) > 1:
                    for w in waits[:-1]:
                        nop = mybir.InstNoOp(
                            name=f"wsplit-{n_extra}-{inst.name}",
                            engine=inst.engine,
                            bass_nofuse=True,
                            sync_info=mybir.SyncInfo(on_wait=[w], on_update=[]),
                        )
                        new.append(nop)
                        n_extra += 1
                    si.on_wait = [waits[-1]]
                new.append(inst)
            if n_extra:
                bb.instructions[:] = new
    return n_extra


def build_nc(n_steps=T, split_waits=True):
    import concourse.bass as bass
    from concourse import mybir
    from contextlib import ExitStack

    FP = mybir.dt.float32
    FPR = mybir.dt.float32r
    BF = mybir.dt.bfloat16
    AF = mybir.ActivationFunctionType
    ALU = mybir.AluOpType
    nc = bass.Bass()

    def mm(out, lhsT, rhs, start, stop):
        nc.tensor.matmul(out, lhsT, rhs, start=start, stop=stop)

    # ---- DRAM parameters (host-prepped layouts) ----
    dp = lambda name, shape, dt=FP: nc.declare_dram_parameter(name, list(shape), dt, isOutput=False)
    xT_d = dp("xT", (128, 5 * BC), BF)
    bW_d = [dp("bW0", (128, 5 * HID), BF)] + [dp(f"bW{i}", (128, 4 * HID), BF) for i in (1, 2, 3)]
    bb_d = [dp(f"bb{i}", (128, 4)) for i in range(4)]
    tW0_d = dp("tW0", (1, HID), BF)
    tW_d = [None] + [dp(f"tW{i}", (128, 4 * HID), BF) for i in (1, 2, 3)]
    tb_d = [dp(f"tb{i}", (128, 4)) for i in range(4)]
    tT_d = dp("tT", (1, T), BF)
    # GRU transposed lhsT weights: [k-tile, gate-Mtile] blocks of [128, 128]
    whh0T_d = dp("whh0T", (128, 12 * 128), BF)
    whh1T_d = dp("whh1T", (128, 12 * 128), BF)
    wih1T_d = dp("wih1T", (128, 12 * 128), BF)
    a0_d = dp("a0", (2, 768), BF)        # [w0_g; bias_g] per gate block (rz + n-in)
    b1rz_d = dp("b1rz", (1, 512), BF)    # bih1+bhh1 rz
    bhh0n_d = dp("bhh0n", (1, 256), BF)
    bhh1n_d = dp("bhh1n", (1, 256), BF)
    bih1n_d = dp("bih1n", (1, 256), BF)
    pWT_d = dp("pWT", (2, 128, NS), BF)
    pb_d = dp("pb", (1, NS), BF)
    ident_d = dp("ident", (128, 128))
    ones16k_d = dp("ones16k", (1, T * BC), BF)
    out_d = nc.declare_dram_parameter("out", [BC, NS], FP, isOutput=True)

    with ExitStack() as ctx:
        tc = ctx.enter_context(_patched_tile_context(nc))
        const = ctx.enter_context(tc.tile_pool(name="const", bufs=1))

        # ---- persistent SBUF ----
        ident = const.tile([128, 128], FP)
        nc.gpsimd.dma_start(ident[:], ident_d[:])
        whh0T = const.tile([128, 12 * 128], BF)
        whh1T = const.tile([128, 12 * 128], BF)
        wih1T = const.tile([128, 12 * 128], BF)
        nc.scalar.dma_start(whh0T[:], whh0T_d[:])
        nc.sync.dma_start(whh1T[:], whh1T_d[:])
        nc.sync.dma_start(wih1T[:], wih1T_d[:])
        a0 = const.tile([2, 768], BF)
        nc.gpsimd.dma_start(a0[:], a0_d[:])
        b1rz = const.tile([1, 512], BF)
        nc.gpsimd.dma_start(b1rz[:], b1rz_d[:])
        bhh0n = const.tile([1, 256], BF)
        nc.gpsimd.dma_start(bhh0n[:], bhh0n_d[:])
        bhh1n = const.tile([1, 256], BF)
        nc.gpsimd.dma_start(bhh1n[:], bhh1n_d[:])
        bih1n = const.tile([1, 256], BF)
        nc.gpsimd.dma_start(bih1n[:], bih1n_d[:])
        pWT = const.tile([128, 2 * NS], BF)
        for k in range(2):
            nc.gpsimd.dma_start(pWT[:, k * NS:(k + 1) * NS], pWT_d[k])
        pb = const.tile([1, NS], BF)
        nc.gpsimd.dma_start(pb[:], pb_d[:])
        ones1 = const.tile([1, 128], BF)
        nc.gpsimd.dma_start(ones1[:], ones16k_d[:, 0:128])

        branchT = const.tile([128, HID], BF)  # [feat within tile, 4 mtiles * batch]
        trunkT = const.tile([128, HID], BF)
        seq_sb = const.tile([BC, T], FP)       # [batch, t]
        seqT_sb = const.tile([T, BC], BF)      # [t, batch]

        # GRU state in [feat, batch] layout. h0 double-buffered (L1 runs two
        # steps behind L0 and needs h0(t-2) intact); h1 single.
        h0s = [const.tile([128, 256], BF, name=f'h0_{i}') for i in range(2)]
        h1 = const.tile([128, 256], BF)
        nc.vector.memset(h0s[0][:], 0.0)
        nc.vector.memset(h0s[1][:], 0.0)
        nc.vector.memset(h1[:], 0.0)
        ones_bb = const.tile([128, 256], BF)
        nc.vector.memset(ones_bb[:], 1.0)
        zrow = const.tile([1, 128], BF)
        nc.vector.memset(zrow[:], 0.0)
        u0_sb = const.tile([128, 256], BF)
        u1_sb = const.tile([128, 256], BF)
        zc0 = const.tile([128, 256], BF)
        zc1 = const.tile([128, 256], BF)

        # gate tiles
        rz0 = const.tile([128, 512], BF)
        rz1 = const.tile([128, 512], BF)
        n0 = const.tile([128, 256], BF)
        n1 = const.tile([128, 256], BF)
        t1b_0 = const.tile([128, 256], BF)
        t2b_0 = const.tile([128, 256], BF)
        t1_1 = const.tile([128, 256], BF)
        t2_1 = const.tile([128, 256], BF)
        zh0 = const.tile([128, 256], BF)
        zh1 = const.tile([128, 256], BF)
        m0 = const.tile([128, 256], BF)
        m1 = const.tile([128, 256], BF)

        # ================= MLP phase (as baseline) =================
        with tc.tile_pool(name="mlpw", bufs=1) as mlpw, \
             tc.tile_pool(name="mlps", bufs=2) as mlps, \
             tc.tile_pool(name="mlpp", bufs=4, space=bass.MemorySpace.PSUM) as mlpp:

            def mlp(xtiles_sb, nk_first, W_sbs, b_sbs, final_relu, out_sb):
                cur = xtiles_sb
                nlayers = 4
                for l in range(nlayers):
                    nk = nk_first if l == 0 else 4
                    Wl = W_sbs[l]
                    dst = out_sb if l == nlayers - 1 else mlps.tile([128, HID], BF, tag="mlpact")
                    for m in range(4):
                        ps = mlpp.tile([128, 128], FP, tag="mlppsum")
                        for k in range(nk):
                            mm(
                                ps[:],
                                Wl[:, k * HID + m * 128: k * HID + (m + 1) * 128],
                                cur[:, k * 128:(k + 1) * 128],
                                start=(k == 0), stop=(k == nk - 1),
                            )
                        func = AF.Relu if (l < nlayers - 1 or final_relu) else AF.Identity
                        nc.scalar.activation(
                            dst[:, m * 128:(m + 1) * 128], ps[:], func,
                            bias=b_sbs[l][:, m:m + 1],
                        )
                    cur = dst
                return cur

            bW_sb = []
            dma_engs = [nc.gpsimd, nc.scalar, nc.sync, nc.gpsimd]
            for l in range(4):
                nk = 5 if l == 0 else 4
                w = mlpw.tile([128, nk * HID], BF, tag=f"bw{l}")
                dma_engs[l % 4].dma_start(w[:], bW_d[l][:])
                bW_sb.append(w)
            bb_sb = []
            for l in range(4):
                t_ = mlpw.tile([128, 4], FP, tag=f"bb{l}")
                nc.gpsimd.dma_start(t_[:], bb_d[l][:])
                bb_sb.append(t_)
            xk = mlpw.tile([128, 5 * 128], BF, tag="xk")
            nc.gpsimd.dma_start(xk[:], xT_d[:])
            mlp(xk, 5, bW_sb, bb_sb, final_relu=False, out_sb=branchT)

            tW0 = mlpw.tile([1, HID], BF, tag="tw0")
            nc.gpsimd.dma_start(tW0[:], tW0_d[:])
            tTs = mlpw.tile([1, T], BF, tag="tts")
            nc.gpsimd.dma_start(tTs[:], tT_d[:])
            tb_sb = []
            for l in range(4):
                t_ = mlpw.tile([128, 4], FP, tag=f"tb{l}")
                nc.gpsimd.dma_start(t_[:], tb_d[l][:])
                tb_sb.append(t_)
            tW_sb = [None]
            for l in (1, 2, 3):
                w = mlpw.tile([128, 4 * HID], BF, tag=f"tw{l}")
                dma_engs[l % 4].dma_start(w[:], tW_d[l][:])
                tW_sb.append(w)

            tact = mlps.tile([128, HID], BF, tag="mlpact")
            for m in range(4):
                ps = mlpp.tile([128, 128], FP, tag="mlppsum")
                mm(ps[:], tW0[:, m * 128:(m + 1) * 128], tTs[:],
                   start=True, stop=True)
                nc.scalar.activation(tact[:, m * 128:(m + 1) * 128], ps[:],
                                     AF.Relu, bias=tb_sb[0][:, m:m + 1])
            cur = tact
            for l in (1, 2, 3):
                dst = trunkT if l == 3 else mlps.tile([128, HID], BF, tag="mlpact")
                for m in range(4):
                    ps = mlpp.tile([128, 128], FP, tag="mlppsum")
                    for k in range(4):
                        mm(
                            ps[:],
                            tW_sb[l][:, k * HID + m * 128: k * HID + (m + 1) * 128],
                            cur[:, k * 128:(k + 1) * 128],
                            start=(k == 0), stop=(k == 3),
                        )
                    nc.scalar.activation(dst[:, m * 128:(m + 1) * 128], ps[:],
                                         AF.Relu, bias=tb_sb[l][:, m:m + 1])
                cur = dst

            # seq[b,t] = sum_f branchT[f,b] * trunkT[f,t]
            ps_seq = mlpp.tile([128, 128], FP, tag="mlppsum")
            for k in range(4):
                mm(ps_seq[:], branchT[:, k * 128:(k + 1) * 128],
                   trunkT[:, k * 128:(k + 1) * 128],
                   start=(k == 0), stop=(k == 3))
            nc.scalar.copy(seq_sb[:], ps_seq[:])
            ps_seqT = mlpp.tile([128, 128], FP, tag="mlppsum")
            nc.tensor.transpose(ps_seqT[:], seq_sb[:], ident[:])
            nc.scalar.copy(seqT_sb[:], ps_seqT[:])

        # ================= GRU phase =================
        saug = const.tile([2, T * BC], BF)
        nc.gpsimd.dma_start(saug[0:1, :], seqT_sb[:])
        nc.gpsimd.dma_start(saug[1:2, :], ones16k_d[:])

        # whh0T/whh1T/wih1T block accessor: k in {0,1}, gate block g in 0..5
        # (g: 0=r_f0 1=r_f1 2=z_f0 3=z_f1 4=n_f0 5=n_f1)
        def wblk(w, k, g):
            i = k * 6 + g
            return w[:, i * 128:(i + 1) * 128]

        with tc.tile_pool(name="gp0rz", bufs=1, space=bass.MemorySpace.PSUM) as gp0rz, \
             tc.tile_pool(name="gp0nn", bufs=2, space=bass.MemorySpace.PSUM) as gp0nn, \
             tc.tile_pool(name="gp1rz", bufs=2, space=bass.MemorySpace.PSUM) as gp1rz, \
             tc.tile_pool(name="gp1nn", bufs=3, space=bass.MemorySpace.PSUM) as gp1nn:

            # PSUM bank discipline: the FIRST matmul into a bank each
            # iteration has start=True (hardware zeroes the whole 2KB bank),
            # every other matmul accumulates (start=False), the last one
            # carries stop=True. Regions interleave freely in between.
            # No-state-dep matmuls (seq inputs, biases) lead the stream so
            # the PE has work while the previous iteration's gate chain runs.

            def l0_open(t):
                st = saug[:, t * BC:(t + 1) * BC]          # [2, 128]: seq row, ones
                ones_r = ones1[:]                          # [1, 128]
                P0rz = gp0rz.tile([128, 512], FP, tag="P0rz")
                P0nn = gp0nn.tile([128, 512], FP, tag="P0nn")  # [n_rec(256) | n_in(256)]
                # P0nn bank: n-input (seq-only) opens, then bhh0n biases
                for j in range(2):
                    mm(P0nn[:, 256 + j * 128:256 + (j + 1) * 128],
                       a0[:, 512 + j * 128:512 + (j + 1) * 128], st,
                       start=(j == 0), stop=False)
                for j in range(2):
                    mm(P0nn[:, j * 128:(j + 1) * 128],
                       bhh0n[:, j * 128:(j + 1) * 128], ones_r,
                       start=False, stop=False)
                # P0rz bank: seq-input mms open (they carry all rz biases)
                for g in range(4):
                    mm(P0rz[:, g * 128:(g + 1) * 128],
                       a0[:, g * 128:(g + 1) * 128], st,
                       start=(g == 0), stop=False)
                return P0rz, P0nn

            def l0_pad(P0nn):
                # PE clock keep-alive: accumulate zeros while waiting for h0
                for _ in range(N_PAD):
                    mm(P0nn[:, 256:384], zrow[:], ones1[:], start=False, stop=False)

            def l0_dep(t, P0rz, P0nn):
                hp = h0s[(t - 1) % 2]                      # h0(t-1)
                # r first, then n, then z
                for g in (0, 1):
                    o = P0rz[:, g * 128:(g + 1) * 128]
                    mm(o, wblk(whh0T, 0, g), hp[:, 0:128], start=False, stop=False)
                    mm(o, wblk(whh0T, 1, g), hp[:, 128:256], start=False, stop=False)
                for j in range(2):
                    o = P0nn[:, j * 128:(j + 1) * 128]
                    mm(o, wblk(whh0T, 0, 4 + j), hp[:, 0:128], start=False, stop=False)
                    mm(o, wblk(whh0T, 1, 4 + j), hp[:, 128:256],
                       start=False, stop=(j == 1))
                for g in (2, 3):
                    o = P0rz[:, g * 128:(g + 1) * 128]
                    mm(o, wblk(whh0T, 0, g), hp[:, 0:128], start=False, stop=False)
                    mm(o, wblk(whh0T, 1, g), hp[:, 128:256],
                       start=False, stop=(g == 3))

            def l1_open_in(t):
                ones_r = ones1[:]
                hin = h0s[t % 2]                           # h0(t), two iters old
                P1rz = gp1rz.tile([128, 512], FP, tag="P1rz")
                P1nn = gp1nn.tile([128, 512], FP, tag="P1nn")
                # --- no-dep openers: biases ---
                for g in range(4):
                    mm(P1rz[:, g * 128:(g + 1) * 128],
                       b1rz[:, g * 128:(g + 1) * 128], ones_r,
                       start=(g == 0), stop=False)
                for j in range(2):
                    mm(P1nn[:, 256 + j * 128:256 + (j + 1) * 128],
                       bih1n[:, j * 128:(j + 1) * 128], ones_r,
                       start=(j == 0), stop=False)
                for j in range(2):
                    mm(P1nn[:, j * 128:(j + 1) * 128],
                       bhh1n[:, j * 128:(j + 1) * 128], ones_r,
                       start=False, stop=False)
                # --- h0-dep: input side (h0(t), two iterations old) ---
                for g in range(4):
                    o = P1rz[:, g * 128:(g + 1) * 128]
                    mm(o, wblk(wih1T, 0, g), hin[:, 0:128], start=False, stop=False)
                    mm(o, wblk(wih1T, 1, g), hin[:, 128:256], start=False, stop=False)
                for j in range(2):
                    o = P1nn[:, 256 + j * 128:256 + (j + 1) * 128]
                    mm(o, wblk(wih1T, 0, 4 + j), hin[:, 0:128], start=False, stop=False)
                    mm(o, wblk(wih1T, 1, 4 + j), hin[:, 128:256], start=False, stop=False)
                return P1rz, P1nn

            def l1_rec(P1rz, P1nn):
                # h1(t-1): produced by the previous iteration's L1 chain,
                # consumed here at the very END of this iteration's stream
                for g in range(4):
                    o = P1rz[:, g * 128:(g + 1) * 128]
                    mm(o, wblk(whh1T, 0, g), h1[:, 0:128], start=False, stop=False)
                    mm(o, wblk(whh1T, 1, g), h1[:, 128:256],
                       start=False, stop=(g == 3))
                for j in range(2):
                    o = P1nn[:, j * 128:(j + 1) * 128]
                    mm(o, wblk(whh1T, 0, 4 + j), h1[:, 0:128], start=False, stop=False)
                    mm(o, wblk(whh1T, 1, 4 + j), h1[:, 128:256],
                       start=False, stop=(j == 1))

            def l0_evac(P0nn):
                # n-input sits in PSUM right after the openers; move it to
                # SBUF bf16 on the idle Act engine so t2 is a cheap bf16 add
                nc.scalar.copy(u0_sb[:], P0nn[:, 256:512])

            def l0_gates(t, P0rz, P0nn):
                hp = h0s[(t - 1) % 2]
                hc = h0s[t % 2]
                # r before z: r unblocks the n-chain
                nc.scalar.activation(rz0[:, 0:256], P0rz[:, 0:256], AF.Sigmoid)
                nc.scalar.activation(rz0[:, 256:512], P0rz[:, 256:512], AF.Sigmoid)
                nc.vector.tensor_tensor(t1b_0[:], rz0[:, 0:256], P0nn[:, 0:256],
                                        op=ALU.mult)
                nc.vector.tensor_tensor(t2b_0[:], t1b_0[:], u0_sb[:],
                                        op=ALU.add)
                # zh0 = z0*h0 off critical path on Pool
                nc.gpsimd.tensor_tensor(zh0[:], rz0[:, 256:512], hp[:],
                                        op=ALU.mult)
                nc.scalar.activation(n0[:], t2b_0[:], AF.Tanh)
                # m0 = (z0-1)*n0 ; h0 = zh0 - m0 = z*h + (1-z)*n
                nc.vector.scalar_tensor_tensor(m0[:], rz0[:, 256:512], 1.0, n0[:],
                                               op0=ALU.subtract, op1=ALU.mult)
                nc.vector.tensor_tensor(hc[:], zh0[:], m0[:], op=ALU.subtract)

            def l1_gates(t, P1rz, P1nn):
                nc.scalar.activation(rz1[:], P1rz[:], AF.Sigmoid)
                nc.vector.tensor_tensor(t1_1[:], rz1[:, 0:256], P1nn[:, 0:256],
                                        op=ALU.mult)
                nc.vector.tensor_tensor(t2_1[:], t1_1[:], u1_sb[:],
                                        op=ALU.add)
                nc.gpsimd.tensor_tensor(zh1[:], rz1[:, 256:512], h1[:],
                                        op=ALU.mult)
                nc.scalar.activation(n1[:], t2_1[:], AF.Tanh)
                nc.vector.scalar_tensor_tensor(m1[:], rz1[:, 256:512], 1.0, n1[:],
                                               op0=ALU.subtract, op1=ALU.mult)
                nc.vector.tensor_tensor(h1[:], zh1[:], m1[:], op=ALU.subtract)

            # L1 matmuls two steps behind L0, L1 GATES three steps behind:
            # an iteration's L1 gate ops read PSUM produced in the PREVIOUS
            # iteration, so they are ready at iteration start and are issued
            # first — no engine ever waits on the current PE stream tail.
            pendP = None     # (P1rz, P1nn) of the previous iteration
            for t in range(n_steps):
                P0rz, P0nn = l0_open(t)
                if pendP is not None:
                    with tc.high_priority(offset=400):
                        l1_gates(t - 3, *pendP)
                    pendP = None
                if t >= 2:
                    P1rz, P1nn = l1_open_in(t - 2)
                    with tc.high_priority(offset=200):
                        nc.scalar.copy(u1_sb[:], P1nn[:, 256:512])
                l0_pad(P0nn)
                l0_dep(t, P0rz, P0nn)
                if t >= 2:
                    l1_rec(P1rz, P1nn)
                    pendP = (P1rz, P1nn)
                with tc.high_priority(offset=200):
                    l0_evac(P0nn)
                l0_gates(t, P0rz, P0nn)
            # flush: gates for step T-3 (pending), then steps T-2, T-1
            if pendP is not None:
                l1_gates(n_steps - 3, *pendP)
            for ts_ in (n_steps - 2, n_steps - 1):
                if ts_ < 0:
                    continue
                P1rz, P1nn = l1_open_in(ts_)
                nc.scalar.copy(u1_sb[:], P1nn[:, 256:512])
                l1_rec(P1rz, P1nn)
                l1_gates(ts_, P1rz, P1nn)

            # ---- projection: out[b, s] from h1 in [feat, batch] layout ----
            Pout = gp0rz.tile([128, 512], FP, tag="P0rz")
            mm(Pout[:, 0:NS], h1[:, 0:128], pWT[:, 0:NS], start=True, stop=False)
            mm(Pout[:, 0:NS], h1[:, 128:256], pWT[:, NS:2 * NS], start=False, stop=False)
            mm(Pout[:, 0:NS], ones1[:], pb[:], start=False, stop=True)
            out_sb = const.tile([128, NS], FP)
            nc.scalar.copy(out_sb[:], Pout[:, 0:NS])
            nc.gpsimd.dma_start(out_d[:], out_sb[:])

    if split_waits:
        _split_multi_waits(nc)
    return nc


def prep_inputs(inputs):
    """Host-side shared (per-core-identical) tensor prep."""
    f = np.float32
    import ml_dtypes
    bf = ml_dtypes.bfloat16
    g = {}
    bWf = np.asarray(inputs['branch_Wf'], f)      # (512, 528)
    bWr = np.asarray(inputs['branch_Wr'], f)      # (3, 512, 512)
    w = np.zeros((NBP, HID), f)
    w[:NB] = bWf.T
    g['bW0'] = np.concatenate([w.reshape(5, 128, HID)[k] for k in range(5)], axis=1).astype(bf)
    for i in range(3):
        bk = np.ascontiguousarray(bWr[i].T).reshape(4, 128, HID)
        g[f'bW{i + 1}'] = np.concatenate([bk[k] for k in range(4)], axis=1).astype(bf)
    g['bb0'] = np.asarray(inputs['branch_bf'], f).reshape(4, 128).T.copy()
    for i in range(3):
        g[f'bb{i + 1}'] = np.asarray(inputs['branch_br'][i], f).reshape(4, 128).T.copy()
    g['tW0'] = np.asarray(inputs['trunk_Wf'], f).T.astype(bf)      # (1, 512)
    tWr = np.asarray(inputs['trunk_Wr'], f)
    for i in range(3):
        tk = np.ascontiguousarray(tWr[i].T).reshape(4, 128, HID)
        g[f'tW{i + 1}'] = np.concatenate([tk[k] for k in range(4)], axis=1).astype(bf)
    g['tb0'] = np.asarray(inputs['trunk_bf'], f).reshape(4, 128).T.copy()
    for i in range(3):
        g[f'tb{i + 1}'] = np.asarray(inputs['trunk_br'][i], f).reshape(4, 128).T.copy()
    g['tT'] = np.arange(T, dtype=f).reshape(1, T).astype(bf)

    # GRU weights, transposed lhsT blocks [k-tile(2) x gate-block(6)]
    def lhsT_blocks(W):
        # W: [768, 256] (torch [3H, H]); block (k, g) = W[g*128:(g+1)*128,
        # k*128:(k+1)*128].T -> [128 (k-part), 128 (m-free)]; blocks
        # concatenated along the free dim in (k, g) order.
        W = np.asarray(W, f)
        out = np.zeros((128, 12 * 128), f)
        for k in range(2):
            for gi in range(6):
                i = k * 6 + gi
                out[:, i * 128:(i + 1) * 128] = W[gi * 128:(gi + 1) * 128,
                                                  k * 128:(k + 1) * 128].T
        return out.astype(bf)

    g['whh0T'] = lhsT_blocks(inputs['gru_Whh0'])
    g['whh1T'] = lhsT_blocks(inputs['gru_Whh1'])
    g['wih1T'] = lhsT_blocks(inputs['gru_Wih1'])

    don = float(np.asarray(inputs['don_bias'], f).reshape(-1)[0])
    w0 = np.asarray(inputs['gru_Wih0'], f)[:, 0]  # (768,)
    bih0 = np.asarray(inputs['gru_bih0'], f)
    bhh0 = np.asarray(inputs['gru_bhh0'], f)
    # a0: [2, 768]: cols 0:512 rz gates: row0 = w0_rz, row1 = bih0+bhh0+don*w0
    #              cols 512:768 n-in:   row0 = w0_n,  row1 = bih0_n+don*w0_n
    a0 = np.zeros((2, 768), f)
    a0[0, 0:512] = w0[:512]
    a0[1, 0:512] = bih0[:512] + bhh0[:512] + don * w0[:512]
    a0[0, 512:768] = w0[512:768]
    a0[1, 512:768] = bih0[512:768] + don * w0[512:768]
    g['a0'] = a0.astype(bf)
    g['bhh0n'] = bhh0[512:768].reshape(1, 256).astype(bf)
    bih1 = np.asarray(inputs['gru_bih1'], f)
    bhh1 = np.asarray(inputs['gru_bhh1'], f)
    g['b1rz'] = (bih1[:512] + bhh1[:512]).reshape(1, 512).astype(bf)
    g['bhh1n'] = bhh1[512:768].reshape(1, 256).astype(bf)
    g['bih1n'] = bih1[512:768].reshape(1, 256).astype(bf)
    pW = np.asarray(inputs['proj_W'], f)          # (16, 256)
    pWT = np.zeros((2, 128, NS), f)
    for k in range(2):
        pWT[k] = pW[:, k * 128:(k + 1) * 128].T
    g['pWT'] = pWT.astype(bf)
    g['pb'] = np.asarray(inputs['proj_b'], f).reshape(1, NS).astype(bf)
    g['ident'] = np.eye(128, dtype=f)
    g['ones16k'] = np.ones((1, T * BC), bf)
    return g


def run(inputs, **spmd_kwargs):
    from concourse.bass_utils import run_bass_kernel_spmd

    if 'nc' not in _CACHE:
        _CACHE['nc'] = build_nc(T)
    nc = _CACHE['nc']

    shared = prep_inputs(inputs)
    x = np.asarray(inputs['x'], np.float32)
    in_maps = []
    for c in range(NCORES):
        xs = x[c * BC:(c + 1) * BC]          # (128, 528)
        xt = np.zeros((NBP, BC), np.float32)
        xt[:NB] = xs.T
        m = dict(shared)
        import ml_dtypes
        xk5 = xt.reshape(5, 128, BC)
        m['xT'] = np.concatenate([xk5[k] for k in range(5)], axis=1).astype(ml_dtypes.bfloat16)
        in_maps.append(m)

    res = run_bass_kernel_spmd(nc, in_maps, list(range(NCORES)), **spmd_kwargs)
    out = np.concatenate([res.results[c]["out"] for c in range(NCORES)], axis=0)
    return out.astype(np.float32), res


def kernel(**inputs):
    out, _ = run(inputs)
    return out


if __name__ == "__main__":
    print("building nc...")
    nc = build_nc(2)
    print("built OK")



# revision 5
# speedup vs baseline: 1.0173x; 1.0173x over previous
"""DeepONet+GRU Trainium2 kernel v2 (8-core data parallel).

Full inputs in, full outputs out. Batch 1024 sharded 128/core; params
replicated. Per core:
  branch MLP (528->512 x4) -> branchT [512f, 128b]   (as baseline)
  trunk MLP (1->512 x4)    -> trunkT  [512f, 128t]
  seq = branchT.T @ trunkT -> saug rows (t-major, bf16)
  2-layer GRU scan, hidden 256, computed entirely in TRANSPOSED
  [feature, batch] layout: state h01 [128, 512] bf16 holds
  [h0_f0 | h0_f1 | h1_f0 | h1_f1]. Recurrent/input weights are
  pre-transposed host-side into matmul lhsT blocks, so each step is a
  flat stream of [128,128] matmuls with no PE transposes and no
  PSUM->SBUF state copies; gate math uses bf16 DVE ops (2x/4x modes)
  and per-step biases ride the K=2/K=1 matmuls.
  L1 runs one step behind L0 so its chain overlaps L0's.
"""
import sys
sys.path.insert(0, '/opt/trn_rl_repo')

import numpy as np

B = 1024
BC = 128          # batch per core
NB = 528
NBP = 640         # padded branch input (5 k-tiles)
HID = 512
GH = 256
T = 128
NS = 16
NCORES = 8
import os
N_PAD = int(os.environ.get('GRU_NPAD', '6'))
N_PAD1 = int(os.environ.get('GRU_NPAD1', '3'))
B0RZ = int(os.environ.get('GRU_B0RZ', '2'))
B1NN = int(os.environ.get('GRU_B1NN', '2'))

_CACHE = {}


def _patched_tile_context(nc):
    """TileContext whose tail drain splits sem waits (walrus CoreV3 rejects
    >1 sync wait on a Drain)."""
    import concourse.tile as tile
    from concourse.vector_clock import ScopedClock

    class PatchedTileContext(tile.TileContext):
        def _drain_and_barrier(self, tick_clock, wait_clock):
            nc = self.nc
            drain_inst = nc.sync.drain()
            wait_clock.add_sem_waits(
                drain_inst.ins, ScopedClock({None: tick_clock.global_clock})
            )
            si = drain_inst.ins.sync_info
            waits = list(si.on_wait or []) if si is not None else []
            if len(waits) > 1:
                si.on_wait = waits[:1]
                for i in range(1, len(waits)):
                    extra = nc.sync.drain()
                    esi = extra.ins.sync_info
                    if esi is None:
                        from concourse import mybir
                        extra.ins.sync_info = mybir.SyncInfo(
                            on_wait=waits[i:i + 1], on_update=[]
                        )
                    else:
                        esi.on_wait = waits[i:i + 1]
            nc.all_engine_barrier()
            assert self.sems is not None
            popped = nc._tile_sem_poison_stack.pop()
            assert popped is self._sem_poison
            nc.clear_and_free_semaphores(list(self.sems.allocated().values()))
            nc.all_engine_barrier()

    return PatchedTileContext(nc)


def _split_multi_waits(nc):
    """This container's walrus rejects >1 sync wait per instruction.
    Hoist extra waits onto engine-matched NoOps spliced immediately before
    the offending instruction."""
    from concourse import mybir
    n_extra = 0
    for fn in nc.m.functions:
        for bb in fn.blocks:
            new = []
            for inst in bb.instructions:
                si = inst.sync_info
                waits = list(si.on_wait) if (si is not None and si.on_wait) else []
                if len(waits) > 1:
                    for w in waits[:-1]:
                        nop = mybir.InstNoOp(
                            name=f"wsplit-{n_extra}-{inst.name}",
                            engine=inst.engine,
                            bass_nofuse=True,
                            sync_info=mybir.SyncInfo(on_wait=[w], on_update=[]),
                        )
                        new.append(nop)
                        n_extra += 1
                    si.on_wait = [waits[-1]]
                new.append(inst)
            if n_extra:
                bb.instructions[:] = new
    return n_extra


def build_nc(n_steps=T, split_waits=True):
    import concourse.bass as bass
    from concourse import mybir
    from contextlib import ExitStack

    FP = mybir.dt.float32
    FPR = mybir.dt.float32r
    BF = mybir.dt.bfloat16
    AF = mybir.ActivationFunctionType
    ALU = mybir.AluOpType
    nc = bass.Bass()

    def mm(out, lhsT, rhs, start, stop):
        nc.tensor.matmul(out, lhsT, rhs, start=start, stop=stop)

    # ---- DRAM parameters (host-prepped layouts) ----
    dp = lambda name, shape, dt=FP: nc.declare_dram_parameter(name, list(shape), dt, isOutput=False)
    xT_d = dp("xT", (128, 5 * BC), BF)
    bW_d = [dp("bW0", (128, 5 * HID), BF)] + [dp(f"bW{i}", (128, 4 * HID), BF) for i in (1, 2, 3)]
    bb_d = [dp(f"bb{i}", (128, 4)) for i in range(4)]
    tW0_d = dp("tW0", (1, HID), BF)
    tW_d = [None] + [dp(f"tW{i}", (128, 4 * HID), BF) for i in (1, 2, 3)]
    tb_d = [dp(f"tb{i}", (128, 4)) for i in range(4)]
    tT_d = dp("tT", (1, T), BF)
    # GRU transposed lhsT weights: [k-tile, gate-Mtile] blocks of [128, 128]
    whh0T_d = dp("whh0T", (128, 12 * 128), BF)
    whh1T_d = dp("whh1T", (128, 12 * 128), BF)
    wih1T_d = dp("wih1T", (128, 12 * 128), BF)
    a0_d = dp("a0", (2, 768), BF)        # [w0_g; bias_g] per gate block (rz + n-in)
    b1rz_d = dp("b1rz", (1, 512), BF)    # bih1+bhh1 rz
    bhh0n_d = dp("bhh0n", (1, 256), BF)
    bhh1n_d = dp("bhh1n", (1, 256), BF)
    bih1n_d = dp("bih1n", (1, 256), BF)
    pWT_d = dp("pWT", (2, 128, NS), BF)
    pb_d = dp("pb", (1, NS), BF)
    ident_d = dp("ident", (128, 128))
    ones16k_d = dp("ones16k", (1, T * BC), BF)
    out_d = nc.declare_dram_parameter("out", [BC, NS], FP, isOutput=True)

    with ExitStack() as ctx:
        tc = ctx.enter_context(_patched_tile_context(nc))
        const = ctx.enter_context(tc.tile_pool(name="const", bufs=1))

        # ---- persistent SBUF ----
        ident = const.tile([128, 128], FP)
        nc.gpsimd.dma_start(ident[:], ident_d[:])
        whh0T = const.tile([128, 12 * 128], BF)
        whh1T = const.tile([128, 12 * 128], BF)
        wih1T = const.tile([128, 12 * 128], BF)
        nc.scalar.dma_start(whh0T[:], whh0T_d[:])
        nc.sync.dma_start(whh1T[:], whh1T_d[:])
        nc.sync.dma_start(wih1T[:], wih1T_d[:])
        a0 = const.tile([2, 768], BF)
        nc.gpsimd.dma_start(a0[:], a0_d[:])
        b1rz = const.tile([1, 512], BF)
        nc.gpsimd.dma_start(b1rz[:], b1rz_d[:])
        bhh0n = const.tile([1, 256], BF)
        nc.gpsimd.dma_start(bhh0n[:], bhh0n_d[:])
        bhh1n = const.tile([1, 256], BF)
        nc.gpsimd.dma_start(bhh1n[:], bhh1n_d[:])
        bih1n = const.tile([1, 256], BF)
        nc.gpsimd.dma_start(bih1n[:], bih1n_d[:])
        pWT = const.tile([128, 2 * NS], BF)
        for k in range(2):
            nc.gpsimd.dma_start(pWT[:, k * NS:(k + 1) * NS], pWT_d[k])
        pb = const.tile([1, NS], BF)
        nc.gpsimd.dma_start(pb[:], pb_d[:])
        ones1 = const.tile([1, 128], BF)
        nc.gpsimd.dma_start(ones1[:], ones16k_d[:, 0:128])

        branchT = const.tile([128, HID], BF)  # [feat within tile, 4 mtiles * batch]
        trunkT = const.tile([128, HID], BF)
        seq_sb = const.tile([BC, T], FP)       # [batch, t]
        seqT_sb = const.tile([T, BC], BF)      # [t, batch]

        # GRU state in [feat, batch] layout. h0 double-buffered (L1 runs two
        # steps behind L0 and needs h0(t-2) intact); h1 single.
        h0s = [const.tile([128, 256], BF, name=f'h0_{i}') for i in range(2)]
        h1 = const.tile([128, 256], BF)
        nc.vector.memset(h0s[0][:], 0.0)
        nc.vector.memset(h0s[1][:], 0.0)
        nc.vector.memset(h1[:], 0.0)
        ones_bb = const.tile([128, 256], BF)
        nc.vector.memset(ones_bb[:], 1.0)
        zrow = const.tile([1, 128], BF)
        nc.vector.memset(zrow[:], 0.0)
        u0_sb = const.tile([128, 256], BF)
        u1_sb = const.tile([128, 256], BF)
        zc0 = const.tile([128, 256], BF)
        zc1 = const.tile([128, 256], BF)

        # gate tiles
        rz0 = const.tile([128, 512], BF)
        rz1 = const.tile([128, 512], BF)
        n0 = const.tile([128, 256], BF)
        n1 = const.tile([128, 256], BF)
        t1b_0 = const.tile([128, 256], BF)
        t2b_0 = const.tile([128, 256], BF)
        t1_1 = const.tile([128, 256], BF)
        t2_1 = const.tile([128, 256], BF)
        zh0 = const.tile([128, 256], BF)
        zh1 = const.tile([128, 256], BF)
        m0 = const.tile([128, 256], BF)
        m1 = const.tile([128, 256], BF)

        # ================= MLP phase (as baseline) =================
        with tc.tile_pool(name="mlpw", bufs=1) as mlpw, \
             tc.tile_pool(name="mlps", bufs=2) as mlps, \
             tc.tile_pool(name="mlpp", bufs=4, space=bass.MemorySpace.PSUM) as mlpp:

            def mlp(xtiles_sb, nk_first, W_sbs, b_sbs, final_relu, out_sb):
                cur = xtiles_sb
                nlayers = 4
                for l in range(nlayers):
                    nk = nk_first if l == 0 else 4
                    Wl = W_sbs[l]
                    dst = out_sb if l == nlayers - 1 else mlps.tile([128, HID], BF, tag="mlpact")
                    for m in range(4):
                        ps = mlpp.tile([128, 128], FP, tag="mlppsum")
                        for k in range(nk):
                            mm(
                                ps[:],
                                Wl[:, k * HID + m * 128: k * HID + (m + 1) * 128],
                                cur[:, k * 128:(k + 1) * 128],
                                start=(k == 0), stop=(k == nk - 1),
                            )
                        func = AF.Relu if (l < nlayers - 1 or final_relu) else AF.Identity
                        nc.scalar.activation(
                            dst[:, m * 128:(m + 1) * 128], ps[:], func,
                            bias=b_sbs[l][:, m:m + 1],
                        )
                    cur = dst
                return cur

            bW_sb = []
            dma_engs = [nc.gpsimd, nc.scalar, nc.sync, nc.gpsimd]
            for l in range(4):
                nk = 5 if l == 0 else 4
                w = mlpw.tile([128, nk * HID], BF, tag=f"bw{l}")
                dma_engs[l % 4].dma_start(w[:], bW_d[l][:])
                bW_sb.append(w)
            bb_sb = []
            for l in range(4):
                t_ = mlpw.tile([128, 4], FP, tag=f"bb{l}")
                nc.gpsimd.dma_start(t_[:], bb_d[l][:])
                bb_sb.append(t_)
            xk = mlpw.tile([128, 5 * 128], BF, tag="xk")
            nc.gpsimd.dma_start(xk[:], xT_d[:])
            mlp(xk, 5, bW_sb, bb_sb, final_relu=False, out_sb=branchT)

            tW0 = mlpw.tile([1, HID], BF, tag="tw0")
            nc.gpsimd.dma_start(tW0[:], tW0_d[:])
            tTs = mlpw.tile([1, T], BF, tag="tts")
            nc.gpsimd.dma_start(tTs[:], tT_d[:])
            tb_sb = []
            for l in range(4):
                t_ = mlpw.tile([128, 4], FP, tag=f"tb{l}")
                nc.gpsimd.dma_start(t_[:], tb_d[l][:])
                tb_sb.append(t_)
            tW_sb = [None]
            for l in (1, 2, 3):
                w = mlpw.tile([128, 4 * HID], BF, tag=f"tw{l}")
                dma_engs[l % 4].dma_start(w[:], tW_d[l][:])
                tW_sb.append(w)

            tact = mlps.tile([128, HID], BF, tag="mlpact")
            for m in range(4):
                ps = mlpp.tile([128, 128], FP, tag="mlppsum")
                mm(ps[:], tW0[:, m * 128:(m + 1) * 128], tTs[:],
                   start=True, stop=True)
                nc.scalar.activation(tact[:, m * 128:(m + 1) * 128], ps[:],
                                     AF.Relu, bias=tb_sb[0][:, m:m + 1])
            cur = tact
            for l in (1, 2, 3):
                dst = trunkT if l == 3 else mlps.tile([128, HID], BF, tag="mlpact")
                for m in range(4):
                    ps = mlpp.tile([128, 128], FP, tag="mlppsum")
                    for k in range(4):
                        mm(
                            ps[:],
                            tW_sb[l][:, k * HID + m * 128: k * HID + (m + 1) * 128],
                            cur[:, k * 128:(k + 1) * 128],
                            start=(k == 0), stop=(k == 3),
                        )
                    nc.scalar.activation(dst[:, m * 128:(m + 1) * 128], ps[:],
                                         AF.Relu, bias=tb_sb[l][:, m:m + 1])
                cur = dst

            # seq[b,t] = sum_f branchT[f,b] * trunkT[f,t]
            ps_seq = mlpp.tile([128, 128], FP, tag="mlppsum")
            for k in range(4):
                mm(ps_seq[:], branchT[:, k * 128:(k + 1) * 128],
                   trunkT[:, k * 128:(k + 1) * 128],
                   start=(k == 0), stop=(k == 3))
            nc.scalar.copy(seq_sb[:], ps_seq[:])
            ps_seqT = mlpp.tile([128, 128], FP, tag="mlppsum")
            nc.tensor.transpose(ps_seqT[:], seq_sb[:], ident[:])
            nc.scalar.copy(seqT_sb[:], ps_seqT[:])

        # ================= GRU phase =================
        saug = const.tile([2, T * BC], BF)
        nc.gpsimd.dma_start(saug[0:1, :], seqT_sb[:])
        nc.gpsimd.dma_start(saug[1:2, :], ones16k_d[:])

        # whh0T/whh1T/wih1T block accessor: k in {0,1}, gate block g in 0..5
        # (g: 0=r_f0 1=r_f1 2=z_f0 3=z_f1 4=n_f0 5=n_f1)
        def wblk(w, k, g):
            i = k * 6 + g
            return w[:, i * 128:(i + 1) * 128]

        with tc.tile_pool(name="gp0rz", bufs=B0RZ, space=bass.MemorySpace.PSUM) as gp0rz, \
             tc.tile_pool(name="gp0nn", bufs=2, space=bass.MemorySpace.PSUM) as gp0nn, \
             tc.tile_pool(name="gp1rz", bufs=2, space=bass.MemorySpace.PSUM) as gp1rz, \
             tc.tile_pool(name="gp1nn", bufs=B1NN, space=bass.MemorySpace.PSUM) as gp1nn:

            # PSUM bank discipline: the FIRST matmul into a bank each
            # iteration has start=True (hardware zeroes the whole 2KB bank),
            # every other matmul accumulates (start=False), the last one
            # carries stop=True. Regions interleave freely in between.
            # No-state-dep matmuls (seq inputs, biases) lead the stream so
            # the PE has work while the previous iteration's gate chain runs.

            def l0_open(t):
                st = saug[:, t * BC:(t + 1) * BC]          # [2, 128]: seq row, ones
                ones_r = ones1[:]                          # [1, 128]
                P0rz = gp0rz.tile([128, 512], FP, tag="P0rz")
                P0nn = gp0nn.tile([128, 512], FP, tag="P0nn")  # [n_rec(256) | n_in(256)]
                # P0nn bank: n-input (seq-only) opens, then bhh0n biases
                for j in range(2):
                    mm(P0nn[:, 256 + j * 128:256 + (j + 1) * 128],
                       a0[:, 512 + j * 128:512 + (j + 1) * 128], st,
                       start=(j == 0), stop=False)
                for j in range(2):
                    mm(P0nn[:, j * 128:(j + 1) * 128],
                       bhh0n[:, j * 128:(j + 1) * 128], ones_r,
                       start=False, stop=False)
                # P0rz bank: seq-input mms open (they carry all rz biases)
                for g in range(4):
                    mm(P0rz[:, g * 128:(g + 1) * 128],
                       a0[:, g * 128:(g + 1) * 128], st,
                       start=(g == 0), stop=False)
                return P0rz, P0nn

            def l0_pad(P0nn):
                # PE clock keep-alive: accumulate zeros while waiting for h0
                for _ in range(N_PAD):
                    mm(P0nn[:, 256:384], zrow[:], ones1[:], start=False, stop=False)

            def l0_dep(t, P0rz, P0nn):
                hp = h0s[(t - 1) % 2]                      # h0(t-1)
                # r first, then n, then z
                for g in (0, 1):
                    o = P0rz[:, g * 128:(g + 1) * 128]
                    mm(o, wblk(whh0T, 0, g), hp[:, 0:128], start=False, stop=False)
                    mm(o, wblk(whh0T, 1, g), hp[:, 128:256], start=False, stop=False)
                for j in range(2):
                    o = P0nn[:, j * 128:(j + 1) * 128]
                    mm(o, wblk(whh0T, 0, 4 + j), hp[:, 0:128], start=False, stop=False)
                    mm(o, wblk(whh0T, 1, 4 + j), hp[:, 128:256],
                       start=False, stop=(j == 1))
                for g in (2, 3):
                    o = P0rz[:, g * 128:(g + 1) * 128]
                    mm(o, wblk(whh0T, 0, g), hp[:, 0:128], start=False, stop=False)
                    mm(o, wblk(whh0T, 1, g), hp[:, 128:256],
                       start=False, stop=(g == 3))

            def l1_open_in(t):
                ones_r = ones1[:]
                hin = h0s[t % 2]                           # h0(t), two iters old
                P1rz = gp1rz.tile([128, 512], FP, tag="P1rz")
                P1nn = gp1nn.tile([128, 512], FP, tag="P1nn")
                # --- no-dep openers: biases ---
                for g in range(4):
                    mm(P1rz[:, g * 128:(g + 1) * 128],
                       b1rz[:, g * 128:(g + 1) * 128], ones_r,
                       start=(g == 0), stop=False)
                for j in range(2):
                    mm(P1nn[:, 256 + j * 128:256 + (j + 1) * 128],
                       bih1n[:, j * 128:(j + 1) * 128], ones_r,
                       start=(j == 0), stop=False)
                for j in range(2):
                    mm(P1nn[:, j * 128:(j + 1) * 128],
                       bhh1n[:, j * 128:(j + 1) * 128], ones_r,
                       start=False, stop=False)
                # --- h0-dep: input side (h0(t), two iterations old) ---
                for g in range(4):
                    o = P1rz[:, g * 128:(g + 1) * 128]
                    mm(o, wblk(wih1T, 0, g), hin[:, 0:128], start=False, stop=False)
                    mm(o, wblk(wih1T, 1, g), hin[:, 128:256], start=False, stop=False)
                for j in range(2):
                    o = P1nn[:, 256 + j * 128:256 + (j + 1) * 128]
                    mm(o, wblk(wih1T, 0, 4 + j), hin[:, 0:128], start=False, stop=False)
                    mm(o, wblk(wih1T, 1, 4 + j), hin[:, 128:256], start=False, stop=False)
                return P1rz, P1nn

            def l1_pad(P1nn):
                # PE clock keep-alive while waiting for h1(t-1)
                for _ in range(N_PAD1):
                    mm(P1nn[:, 384:512], zrow[:], ones1[:], start=False, stop=False)

            def l1_rec(P1rz, P1nn):
                # h1(t-1): produced by the previous iteration's L1 chain,
                # consumed here at the very END of this iteration's stream
                for g in range(4):
                    o = P1rz[:, g * 128:(g + 1) * 128]
                    mm(o, wblk(whh1T, 0, g), h1[:, 0:128], start=False, stop=False)
                    mm(o, wblk(whh1T, 1, g), h1[:, 128:256],
                       start=False, stop=(g == 3))
                for j in range(2):
                    o = P1nn[:, j * 128:(j + 1) * 128]
                    mm(o, wblk(whh1T, 0, 4 + j), h1[:, 0:128], start=False, stop=False)
                    mm(o, wblk(whh1T, 1, 4 + j), h1[:, 128:256],
                       start=False, stop=(j == 1))

            def l0_evac(P0nn):
                # n-input sits in PSUM right after the openers; move it to
                # SBUF bf16 on the idle Act engine so t2 is a cheap bf16 add
                nc.scalar.copy(u0_sb[:], P0nn[:, 256:512])

            def l0_gates(t, P0rz, P0nn):
                hp = h0s[(t - 1) % 2]
                hc = h0s[t % 2]
                # r before z: r unblocks the n-chain
                nc.scalar.activation(rz0[:, 0:256], P0rz[:, 0:256], AF.Sigmoid)
                nc.scalar.activation(rz0[:, 256:512], P0rz[:, 256:512], AF.Sigmoid)
                nc.vector.tensor_tensor(t1b_0[:], rz0[:, 0:256], P0nn[:, 0:256],
                                        op=ALU.mult)
                nc.vector.tensor_tensor(t2b_0[:], t1b_0[:], u0_sb[:],
                                        op=ALU.add)
                # zh0 = z0*h0 off critical path on Pool
                nc.gpsimd.tensor_tensor(zh0[:], rz0[:, 256:512], hp[:],
                                        op=ALU.mult)
                nc.scalar.activation(n0[:], t2b_0[:], AF.Tanh)
                # m0 = (z0-1)*n0 ; h0 = zh0 - m0 = z*h + (1-z)*n
                nc.vector.scalar_tensor_tensor(m0[:], rz0[:, 256:512], 1.0, n0[:],
                                               op0=ALU.subtract, op1=ALU.mult)
                nc.vector.tensor_tensor(hc[:], zh0[:], m0[:], op=ALU.subtract)

            def l1_gates(t, P1rz, P1nn):
                nc.scalar.activation(rz1[:], P1rz[:], AF.Sigmoid)
                nc.vector.tensor_tensor(t1_1[:], rz1[:, 0:256], P1nn[:, 0:256],
                                        op=ALU.mult)
                nc.vector.tensor_tensor(t2_1[:], t1_1[:], u1_sb[:],
                                        op=ALU.add)
                nc.gpsimd.tensor_tensor(zh1[:], rz1[:, 256:512], h1[:],
                                        op=ALU.mult)
                nc.scalar.activation(n1[:], t2_1[:], AF.Tanh)
                nc.vector.scalar_tensor_tensor(m1[:], rz1[:, 256:512], 1.0, n1[:],
                                               op0=ALU.subtract, op1=ALU.mult)
                nc.vector.tensor_tensor(h1[:], zh1[:], m1[:], op=ALU.subtract)

            # L1 matmuls two steps behind L0, L1 GATES three steps behind:
            # an iteration's L1 gate ops read PSUM produced in the PREVIOUS
            # iteration, so they are ready at iteration start and are issued
            # first — no engine ever waits on the current PE stream tail.
            pendP = None     # (P1rz, P1nn) of the previous iteration
            for t in range(n_steps):
                P0rz, P0nn = l0_open(t)
                if pendP is not None:
                    with tc.high_priority(offset=400):
                        l1_gates(t - 3, *pendP)
                    pendP = None
                if t >= 2:
                    P1rz, P1nn = l1_open_in(t - 2)
                    with tc.high_priority(offset=200):
                        nc.scalar.copy(u1_sb[:], P1nn[:, 256:512])
                l0_pad(P0nn)
                l0_dep(t, P0rz, P0nn)
                if t >= 2:
                    l1_pad(P1nn)
                    l1_rec(P1rz, P1nn)
                    pendP = (P1rz, P1nn)
                with tc.high_priority(offset=200):
                    l0_evac(P0nn)
                l0_gates(t, P0rz, P0nn)
            # flush: gates for step T-3 (pending), then steps T-2, T-1
            if pendP is not None:
                l1_gates(n_steps - 3, *pendP)
            for ts_ in (n_steps - 2, n_steps - 1):
                if ts_ < 0:
                    continue
                P1rz, P1nn = l1_open_in(ts_)
                nc.scalar.copy(u1_sb[:], P1nn[:, 256:512])
                l1_rec(P1rz, P1nn)
                l1_gates(ts_, P1rz, P1nn)

            # ---- projection: out[b, s] from h1 in [feat, batch] layout ----
            Pout = gp0rz.tile([128, 512], FP, tag="P0rz")
            mm(Pout[:, 0:NS], h1[:, 0:128], pWT[:, 0:NS], start=True, stop=False)
            mm(Pout[:, 0:NS], h1[:, 128:256], pWT[:, NS:2 * NS], start=False, stop=False)
            mm(Pout[:, 0:NS], ones1[:], pb[:], start=False, stop=True)
            out_sb = const.tile([128, NS], FP)
            nc.scalar.copy(out_sb[:], Pout[:, 0:NS])
            nc.gpsimd.dma_start(out_d[:], out_sb[:])

    if split_waits:
        _split_multi_waits(nc)
    return nc


def prep_inputs(inputs):
    """Host-side shared (per-core-identical) tensor prep."""
    f = np.float32
    import ml_dtypes
    bf = ml_dtypes.bfloat16
    g = {}
    bWf = np.asarray(inputs['branch_Wf'], f)      # (512, 528)
    bWr = np.asarray(inputs['branch_Wr'], f)      # (3, 512, 512)
    w = np.zeros((NBP, HID), f)
    w[:NB] = bWf.T
    g['bW0'] = np.concatenate([w.reshape(5, 128, HID)[k] for k in range(5)], axis=1).astype(bf)
    for i in range(3):
        bk = np.ascontiguousarray(bWr[i].T).reshape(4, 128, HID)
        g[f'bW{i + 1}'] = np.concatenate([bk[k] for k in range(4)], axis=1).astype(bf)
    g['bb0'] = np.asarray(inputs['branch_bf'], f).reshape(4, 128).T.copy()
    for i in range(3):
        g[f'bb{i + 1}'] = np.asarray(inputs['branch_br'][i], f).reshape(4, 128).T.copy()
    g['tW0'] = np.asarray(inputs['trunk_Wf'], f).T.astype(bf)      # (1, 512)
    tWr = np.asarray(inputs['trunk_Wr'], f)
    for i in range(3):
        tk = np.ascontiguousarray(tWr[i].T).reshape(4, 128, HID)
        g[f'tW{i + 1}'] = np.concatenate([tk[k] for k in range(4)], axis=1).astype(bf)
    g['tb0'] = np.asarray(inputs['trunk_bf'], f).reshape(4, 128).T.copy()
    for i in range(3):
        g[f'tb{i + 1}'] = np.asarray(inputs['trunk_br'][i], f).reshape(4, 128).T.copy()
    g['tT'] = np.arange(T, dtype=f).reshape(1, T).astype(bf)

    # GRU weights, transposed lhsT blocks [k-tile(2) x gate-block(6)]
    def lhsT_blocks(W):
        # W: [768, 256] (torch [3H, H]); block (k, g) = W[g*128:(g+1)*128,
        # k*128:(k+1)*128].T -> [128 (k-part), 128 (m-free)]; blocks
        # concatenated along the free dim in (k, g) order.
        W = np.asarray(W, f)
        out = np.zeros((128, 12 * 128), f)
        for k in range(2):
            for gi in range(6):
                i = k * 6 + gi
                out[:, i * 128:(i + 1) * 128] = W[gi * 128:(gi + 1) * 128,
                                                  k * 128:(k + 1) * 128].T
        return out.astype(bf)

    g['whh0T'] = lhsT_blocks(inputs['gru_Whh0'])
    g['whh1T'] = lhsT_blocks(inputs['gru_Whh1'])
    g['wih1T'] = lhsT_blocks(inputs['gru_Wih1'])

    don = float(np.asarray(inputs['don_bias'], f).reshape(-1)[0])
    w0 = np.asarray(inputs['gru_Wih0'], f)[:, 0]  # (768,)
    bih0 = np.asarray(inputs['gru_bih0'], f)
    bhh0 = np.asarray(inputs['gru_bhh0'], f)
    # a0: [2, 768]: cols 0:512 rz gates: row0 = w0_rz, row1 = bih0+bhh0+don*w0
    #              cols 512:768 n-in:   row0 = w0_n,  row1 = bih0_n+don*w0_n
    a0 = np.zeros((2, 768), f)
    a0[0, 0:512] = w0[:512]
    a0[1, 0:512] = bih0[:512] + bhh0[:512] + don * w0[:512]
    a0[0, 512:768] = w0[512:768]
    a0[1, 512:768] = bih0[512:768] + don * w0[512:768]
    g['a0'] = a0.astype(bf)
    g['bhh0n'] = bhh0[512:768].reshape(1, 256).astype(bf)
    bih1 = np.asarray(inputs['gru_bih1'], f)
    bhh1 = np.asarray(inputs['gru_bhh1'], f)
    g['b1rz'] = (bih1[:512] + bhh1[:512]).reshape(1, 512).astype(bf)
    g['bhh1n'] = bhh1[512:768].reshape(1, 256).astype(bf)
    g['bih1n'] = bih1[512:768].reshape(1, 256).astype(bf)
    pW = np.asarray(inputs['proj_W'], f)          # (16, 256)
    pWT = np.zeros((2, 128, NS), f)
    for k in range(2):
        pWT[k] = pW[:, k * 128:(k + 1) * 128].T
    g['pWT'] = pWT.astype(bf)
    g['pb'] = np.asarray(inputs['proj_b'], f).reshape(1, NS).astype(bf)
    g['ident'] = np.eye(128, dtype=f)
    g['ones16k'] = np.ones((1, T * BC), bf)
    return g


def run(inputs, **spmd_kwargs):
    from concourse.bass_utils import run_bass_kernel_spmd

    if 'nc' not in _CACHE:
        _CACHE['nc'] = build_nc(T)
    nc = _CACHE['nc']

    shared = prep_inputs(inputs)
    x = np.asarray(inputs['x'], np.float32)
    in_maps = []
    for c in range(NCORES):
        xs = x[c * BC:(c + 1) * BC]          # (128, 528)
        xt = np.zeros((NBP, BC), np.float32)
        xt[:NB] = xs.T
        m = dict(shared)
        import ml_dtypes
        xk5 = xt.reshape(5, 128, BC)
        m['xT'] = np.concatenate([xk5[k] for k in range(5)], axis=1).astype(ml_dtypes.bfloat16)
        in_maps.append(m)

    res = run_bass_kernel_spmd(nc, in_maps, list(range(NCORES)), **spmd_kwargs)
    out = np.concatenate([res.results[c]["out"] for c in range(NCORES)], axis=0)
    return out.astype(np.float32), res


def kernel(**inputs):
    out, _ = run(inputs)
    return out


if __name__ == "__main__":
    print("building nc...")
    nc = build_nc(2)
    print("built OK")

